# revision 13
# baseline (speedup 1.0000x reference)
"""Trainium2 Bass kernel for multi-head attention (B=1, N=4096, C=768, H=12, D=64).

Sharding: tensor-parallel over heads across 8 cores. Core c (pair k=c//2):
  even c: head A = 3k   (all 8 query blocks), head B = 3k+1 (query blocks 0-3)
  odd  c: head A = 3k+2 (all 8 query blocks), head B = 3k+1 (query blocks 4-7)
The SPMD program is identical on every core; odd cores receive x^T with its
columns rotated by 2048 so that "local query blocks 0-3" of head B are the
global blocks 4-7.  The host un-permutes rows, normalizes by the softmax row
sums (computed on device via a ones-column appended to V), sums the per-core
partial projections and adds the bias.

On-device layout (per core):
  x^T [768, 4096] fp32 in SBUF; Q^T/K^T computed per head duplicated on both
  partition halves (via host-duplicated weight columns) so consecutive
  score matmuls (contraction=64) can row-tile-pair on the PE array.
  Scores are computed transposed: S^T[m, q] tiles [128, 512] in PSUM,
  exp via ScalarE (scale=1/8 folded in) -> P^T bf16 in SBUF,
  O^T accumulated as V_aug.T @ P^T with V_aug = [V | 1] giving row sums in
  partition row 64.  Final projection per 128-query chunk in fp32r.
"""

import sys

for _p in ("/opt/trn_rl_repo",):
    if _p not in sys.path:
        sys.path.insert(0, _p)

import numpy as np

import concourse.bass as bass  # noqa: F401
import concourse.mybir as mybir
from concourse import bacc, tile
from concourse.bass_utils import run_bass_kernel_spmd

F32 = mybir.dt.float32
F32R = mybir.dt.float32r
BF16 = mybir.dt.bfloat16
AF = mybir.ActivationFunctionType

N = 4096
C = 768
D = 64
NB = 8  # 512-query/key blocks
CC = 6  # 128-row chunks of C
SCALE = D ** -0.5

_NC = None


def _emit(nc, tc, io, ctx):
    xT, w_in, wp_in, y_out, rs_out = (
        io["xT"], io["w"], io["wp"], io["y"], io["rs"])

    sing = ctx.enter_context(tc.tile_pool(name="sing", bufs=1))
    ppsum = ctx.enter_context(tc.tile_pool(name="ppsum", bufs=3, space="PSUM"))
    apsum = ctx.enter_context(tc.tile_pool(name="apsum", bufs=2, space="PSUM"))
    ptp = ctx.enter_context(tc.tile_pool(name="ptp", bufs=6))
    osbp = ctx.enter_context(tc.tile_pool(name="osbp", bufs=2))
    ysbp = ctx.enter_context(tc.tile_pool(name="ysbp", bufs=3))

    # ---- load x^T as 48 [128, 512] tiles (per c-chunk, per n-block) ----
    xt_sb = [[None] * NB for _ in range(CC)]
    for nb in range(NB):
        for cc in range(CC):
            t = sing.tile([128, 512], F32R, name=f"xt_{cc}_{nb}", tag=f"xt_{cc}_{nb}")
            nc.sync.dma_start(out=t, in_=xT[cc * 128:(cc + 1) * 128,
                                            nb * 512:(nb + 1) * 512])
            xt_sb[cc][nb] = t

    # ---- weights: [768, 128] dram -> [128, 6, 128] sbuf (partition = c%128) --
    w_sb = {}
    for name in ("wq_a", "wk_a", "wq_b", "wk_b", "wv"):
        t = sing.tile([128, CC, 128], F32R, name=f"{name}_sb", tag=f"{name}_sb")
        nc.sync.dma_start(out=t, in_=w_in[name].rearrange("(cc p) d -> p cc d", p=128))
        w_sb[name] = t
    wp_sb = {}
    for s, name in ((0, "wp_a"), (1, "wp_b")):
        t = sing.tile([64, C], F32R, name=f"{name}_sb", tag=f"{name}_sb")
        nc.sync.dma_start(out=t, in_=wp_in[s])
        wp_sb[s] = t

    # ---- projection result tiles ----
    KT = [sing.tile([128, N], BF16, name="kt_a", tag="kt_a"),
          sing.tile([128, N], BF16, name="kt_b", tag="kt_b")]
    QT = [sing.tile([128, N], BF16, name="qt_a", tag="qt_a"),
          sing.tile([128, N // 2], BF16, name="qt_b", tag="qt_b")]
    V = [sing.tile([128, 32, 65], BF16, name="v_a", tag="v_a"),
         sing.tile([128, 32, 65], BF16, name="v_b", tag="v_b")]
    # full-tile memset: the ones column at [:, :, 64] survives the data copies
    nc.vector.memset(V[0], 1.0)
    nc.vector.memset(V[1], 1.0)

    def proj(dst, w, nb):
        ps = ppsum.tile([128, 512], F32, name="ps_proj", tag="big")
        for cc in range(CC):
            nc.tensor.matmul(ps, lhsT=w[:, cc, :],
                             rhs=xt_sb[cc][nb],
                             start=(cc == 0), stop=(cc == CC - 1))
        nc.vector.tensor_copy(dst, ps)

    for nb in range(NB):
        proj(KT[0][:, nb * 512:(nb + 1) * 512], w_sb["wk_a"], nb)
        proj(QT[0][:, nb * 512:(nb + 1) * 512], w_sb["wq_a"], nb)
        proj(KT[1][:, nb * 512:(nb + 1) * 512], w_sb["wk_b"], nb)
        if nb < 4:
            proj(QT[1][:, nb * 512:(nb + 1) * 512], w_sb["wq_b"], nb)
        # V directly in [m, dv] orientation: x^T chunk stationary, W_v moving
        for i in range(4):
            mb = nb * 4 + i
            psv = ppsum.tile([128, 128], F32, name="ps_v", tag="big")
            for cc in range(CC):
                nc.tensor.matmul(psv,
                                 lhsT=xt_sb[cc][nb][:, i * 128:(i + 1) * 128],
                                 rhs=w_sb["wv"][:, cc, :],
                                 start=(cc == 0), stop=(cc == CC - 1))
            nc.vector.tensor_copy(V[0][:, mb, 0:64], psv[:, 0:64])
            nc.vector.tensor_copy(V[1][:, mb, 0:64], psv[:, 64:128])

    dbg = io.get("dbg")
    if dbg is not None:
        nc.sync.dma_start(out=dbg["kt_a"], in_=KT[0])
        nc.sync.dma_start(out=dbg["qt_a"], in_=QT[0])
        nc.sync.dma_start(out=dbg["v_a"], in_=V[0])

    # ---- attention: 6 pair-slots, units = (slot, local qb) ----
    pairs = [((0, 0), (1, 0)), ((0, 1), (1, 1)), ((0, 2), (1, 2)),
             ((0, 3), (1, 3)), ((0, 4), (0, 5)), ((0, 6), (0, 7))]
    groups = [list(range(g * 2, g * 2 + 2)) for g in range(16)]

    acc = {}
    for ulo, uup in pairs:
        for u in (ulo, uup):
            acc[u] = apsum.tile([65, 512], F32, name=f"acc_{u[0]}_{u[1]}", tag="acc")

    def emit_av(work):
        (wulo, wuup), mbs, pt = work
        for j, mb in enumerate(mbs):
            for u in (wulo, wuup):
                s, qb = u
                nc.tensor.matmul(
                    acc[u], lhsT=V[s][:, mb, :],
                    rhs=pt[u][:, j * 512:(j + 1) * 512],
                    start=(mb == 0), stop=(mb == 31), skip_group_check=True)

    def emit_finalize(ulo, uup):
        for u in (ulo, uup):
            s, qb = u
            osb = osbp.tile([65, 512], F32R, name="osb", tag="osb")
            nc.vector.tensor_copy(osb, acc[u])
            nc.sync.dma_start(out=rs_out[s][qb:qb + 1, :], in_=osb[64:65, :])
            for qs in range(4):
                py = ppsum.tile([128, C], F32, name="ps_y", tag="big")
                lw = osb[0:64, qs * 128:(qs + 1) * 128]
                nc.tensor.matmul(py[:, 0:512], lhsT=lw,
                                 rhs=wp_sb[s][:, 0:512],
                                 start=True, stop=True)
                nc.tensor.matmul(py[:, 512:C], lhsT=lw,
                                 rhs=wp_sb[s][:, 512:C],
                                 start=True, stop=True)
                ysb = ysbp.tile([128, C], F32, name="ysb", tag="ysb")
                nc.vector.tensor_copy(ysb, py)
                row = qb * 512 + qs * 128
                nc.sync.dma_start(out=y_out[s][row:row + 128, :], in_=ysb)

    # Software-pipelined emission: the PE stream is in-order, so AV(g) (which
    # waits on exp(g)) must come AFTER S^T(g+1) or the PE head-of-line blocks
    # on ScalarE every group.
    pending = None       # (pair, mbs, pt) whose AV is not yet emitted
    done_pair = None     # pair whose last AV was just emitted
    for pair in pairs:
        ulo, uup = pair
        for mbs in groups:
            w = 512 * len(mbs)
            ps = {}
            for u, half in ((ulo, 0), (uup, 64)):
                ps[u] = ppsum.tile([128, 1024], F32, name="ps_s", tag="big")
            for j, mb in enumerate(mbs):
                for u, half in ((ulo, 0), (uup, 64)):
                    s, qb = u
                    nc.tensor.matmul(
                        ps[u][:, j * 512:(j + 1) * 512],
                        lhsT=KT[s][half:half + 64, mb * 128:(mb + 1) * 128],
                        rhs=QT[s][half:half + 64, qb * 512:(qb + 1) * 512],
                        start=True, stop=True)
            pt = {}
            for u in (ulo, uup):
                pt[u] = ptp.tile([128, 1024], BF16, name="pt", tag="pt")
                nc.scalar.activation(out=pt[u][:, :w], in_=ps[u][:, :w],
                                     func=AF.Exp, scale=SCALE)
            if pending is not None:
                emit_av(pending)
                if pending[1][-1] == 31:
                    done_pair = pending[0]
            pending = (pair, mbs, pt)
            if done_pair is not None:
                emit_finalize(*done_pair)
                done_pair = None
    emit_av(pending)
    emit_finalize(*pending[0])


def _build(debug_outputs=False):
    nc = bacc.Bacc("TRN2", debug=False, enable_asserts=False, num_devices=8)
    io = {
        "xT": nc.dram_tensor("xt", [C, N], F32R, kind="ExternalInput").ap(),
        "w": {n: nc.dram_tensor(n, [C, 128], F32R, kind="ExternalInput").ap()
              for n in ("wq_a", "wk_a", "wq_b", "wk_b", "wv")},
        "wp": [nc.dram_tensor("wp_a", [D, C], F32R, kind="ExternalInput").ap(),
               nc.dram_tensor("wp_b", [D, C], F32R, kind="ExternalInput").ap()],
        "y": [nc.dram_tensor("y_a", [N, C], F32, kind="ExternalOutput").ap(),
              nc.dram_tensor("y_b", [N // 2, C], F32, kind="ExternalOutput").ap()],
        "rs": [nc.dram_tensor("rs_a", [NB, 512], F32R, kind="ExternalOutput").ap(),
               nc.dram_tensor("rs_b", [4, 512], F32R, kind="ExternalOutput").ap()],
    }
    if debug_outputs:
        io["dbg"] = {
            "kt_a": nc.dram_tensor("dbg_kt_a", [128, N], BF16,
                                   kind="ExternalOutput").ap(),
            "qt_a": nc.dram_tensor("dbg_qt_a", [128, N], BF16,
                                   kind="ExternalOutput").ap(),
            "v_a": nc.dram_tensor("dbg_v_a", [128, 32, 65], BF16,
                                  kind="ExternalOutput").ap(),
            "pt": nc.dram_tensor("dbg_pt", [11, 128, 1024], BF16,
                                 kind="ExternalOutput").ap(),
        }
    from contextlib import ExitStack
    with tile.TileContext(nc) as tc, ExitStack() as ctx:
        _emit(nc, tc, io, ctx)
    nc.compile()
    return nc


def _get_nc():
    global _NC
    if _NC is None:
        _NC = _build()
    return _NC


def _in_maps(x, W_qkv, W_proj):
    xt_base = np.ascontiguousarray(x[0].T.astype(np.float32))  # [768, 4096]
    rot = np.concatenate([np.arange(2048, 4096), np.arange(0, 2048)])

    def wq(h):
        return W_qkv[h * 64:(h + 1) * 64, :]

    def wk(h):
        return W_qkv[C + h * 64:C + (h + 1) * 64, :]

    def wv(h):
        return W_qkv[2 * C + h * 64:2 * C + (h + 1) * 64, :]

    def dup(m):  # [64, 768] -> [768, 128] with both halves identical
        return np.ascontiguousarray(np.concatenate([m.T, m.T], axis=1))

    maps = []
    for c in range(8):
        k = c // 2
        if c % 2 == 0:
            hA, hB = 3 * k, 3 * k + 1
            xt = xt_base
        else:
            hA, hB = 3 * k + 2, 3 * k + 1
            xt = np.ascontiguousarray(xt_base[:, rot])
        maps.append({
            "xt": xt,
            "wq_a": dup(wq(hA)), "wk_a": dup(wk(hA)),
            "wq_b": dup(wq(hB)), "wk_b": dup(wk(hB)),
            "wv": np.ascontiguousarray(
                np.concatenate([wv(hA).T, wv(hB).T], axis=1)),
            "wp_a": np.ascontiguousarray(W_proj[:, hA * 64:(hA + 1) * 64].T),
            "wp_b": np.ascontiguousarray(W_proj[:, hB * 64:(hB + 1) * 64].T),
        })
    return maps


def kernel(x, xpos, W_qkv, W_proj, b_proj, _results_hook=None):
    x = np.asarray(x, dtype=np.float32)
    W_qkv = np.asarray(W_qkv, dtype=np.float32)
    W_proj = np.asarray(W_proj, dtype=np.float32)
    b_proj = np.asarray(b_proj, dtype=np.float32)

    nc = _get_nc()
    res = run_bass_kernel_spmd(nc, _in_maps(x, W_qkv, W_proj),
                               core_ids=list(range(8)))
    if _results_hook is not None:
        _results_hook(res)

    rot = np.concatenate([np.arange(2048, 4096), np.arange(0, 2048)])
    out = np.zeros((N, C), np.float32)
    for c in range(8):
        r = res.results[c]
        gl = np.arange(N) if c % 2 == 0 else rot
        out[gl] += r["y_a"] / r["rs_a"].reshape(N)[:, None]
        out[gl[:2048]] += r["y_b"] / r["rs_b"].reshape(N // 2)[:, None]
    out += b_proj[None, :]
    return out[None]
